# revision 7
# baseline (speedup 1.0000x reference)
"""Self-contained Trainium2 Bass kernel for single-head full-dim attention.

Reference computation (fp32 jax):
    q  = x @ Wq                      # [B, Nq, D]
    kv = y @ Wkv                     # [B, Nkv, 2D] -> k, v
    attn = softmax(q * D^-0.5 @ k^T) # [B, Nq, Nkv]
    out  = attn @ v                  # [B, Nq, D]
with B=4, Nq=Nkv=2048, D=1024.

Distribution: pure data parallel over 8 NeuronCores; shard = (batch b,
query-half h).  Each core computes q for its 1024 query rows, K/V for the
full 2048 keys of its batch (duplicated across the 2 cores sharing a
batch), the 1024x2048 score block, softmax, and the 1024x1024 output
block.  No collectives.

Layout trick: everything is computed transposed ([feature, token]) so the
TensorEngine can contract along partitions without any on-chip
transposes.  Host pre-transposes x and y, folds the D^-0.5 scale into Wq,
and converts all matmul operands to bf16 (fp32 PSUM accumulation).
Softmax is computed without max-subtraction (scores ~ N(0,1), exp is
safe in fp32) via exp on the scalar engine; the denominator Z is a
ones-vector matmul; normalization multiplies by 1/Z during the PSUM->
SBUF eviction of the output matmul.
"""

import numpy as np
import ml_dtypes

import concourse.bass as bass
import concourse.mybir as mybir
import concourse.tile as tile
from concourse.bass import ds
from concourse.bass_utils import run_bass_kernel_spmd

DIM = 1024
B = 4
NQ = 2048
NKV = 2048
N_CORES = 8
NQ_SHARD = 1024  # query rows per core

BF16 = mybir.dt.bfloat16
F32 = mybir.dt.float32
NP_BF16 = ml_dtypes.bfloat16


def _split_sync_waits(nc, max_waits: int = 1):
    """walrus in this toolchain rejects instructions carrying more than one
    sem wait ("Too many sync wait commands").  Hoist extra waits onto
    preceding same-engine NOPs: the engine dispatches in order, so waiting
    just before the instruction is semantically identical (at worst it
    delays issue slightly)."""
    import bass_rust as _bass_rust

    for f in nc.m.functions:
        for bb in f.blocks:
            insts = list(bb.instructions)
            out = []
            changed = False
            for inst in insts:
                si = getattr(inst, "sync_info", None)
                waits = list(si.on_wait) if si is not None and si.on_wait else []
                if len(waits) > max_waits:
                    changed = True
                    extra, keep = waits[:-max_waits], waits[-max_waits:]
                    for k in range(0, len(extra), max_waits):
                        nop = mybir.InstNoOp(
                            name=f"{inst.name}_sw{k}", engine=inst.engine,
                            ins=[], outs=[],
                        )
                        nop.sync_info = _bass_rust.SyncInfo(
                            on_wait=extra[k : k + max_waits], on_update=[]
                        )
                        out.append(nop)
                    si.on_wait = keep
                    inst.sync_info = si
                out.append(inst)
            if changed:
                bb.instructions = out


def build_attention_nc():
    """Build the per-core Bass graph (identical on all 8 cores)."""
    nc = bass.Bass()

    # DRAM parameters (per-core shards, host-prepped layouts; all bf16
    # except the f32 output).
    xT_d = nc.declare_dram_parameter("xT", [DIM, NQ_SHARD], BF16, isOutput=False)  # [d_in, nq]
    yT_d = nc.declare_dram_parameter("yT", [DIM, NKV], BF16, isOutput=False)  # [d_in, nkv]
    # wq/wk: column slabs: [do_chunk, d_in, 128], slab j = W[:, j*128:(j+1)*128]
    wq_d = nc.declare_dram_parameter("wq", [8, DIM, 128], BF16, isOutput=False)
    wk_d = nc.declare_dram_parameter("wk", [8, DIM, 128], BF16, isOutput=False)
    wv_d = nc.declare_dram_parameter("wv", [DIM, DIM], BF16, isOutput=False)  # [d_in, d_out]
    out_d = nc.declare_dram_parameter("out", [NQ_SHARD, DIM], F32, isOutput=True)

    with tile.TileContext(nc) as tc:
        # Long-lived pool: on-chip intermediates live to the end.
        L = tc.alloc_tile_pool(name="L", bufs=1)
        pm = tc.alloc_tile_pool(name="pm", bufs=1, space="PSUM")
        # Transient input pools, released once consumed.
        t2 = tc.alloc_tile_pool(name="t2", bufs=1)
        t1 = tc.alloc_tile_pool(name="t1", bufs=1)

        # ---- input DMAs -------------------------------------------------
        # 3D tiles [128, chunk, free]: chunk = d_in/128 slab index.
        xt = t1.tile([128, 8, NQ_SHARD], BF16, name="xt", bufs=1)
        nc.sync.dma_start(out=xt[:], in_=xT_d.rearrange("(c p) n -> p c n", p=128))
        yt = t2.tile([128, 8, NKV], BF16, name="yt", bufs=1)
        nc.sync.dma_start(out=yt[:], in_=yT_d.rearrange("(c p) n -> p c n", p=128))
        wv = t2.tile([128, 8, DIM], BF16, name="wv", bufs=1)
        nc.sync.dma_start(out=wv[:], in_=wv_d.rearrange("(c p) n -> p c n", p=128))

        # ---- P1: qT[do, nq] = sum_di Wq_s[di, do] * xT[di, nq] ----------
        qt = [L.tile([128, NQ_SHARD], BF16, name=f"qt{j}", tag="qt", bufs=8) for j in range(8)]
        for j in range(8):  # d_out chunk
            slab = t1.tile([128, 8, 128], BF16, name=f"wq{j}", tag="wq", bufs=3)
            nc.sync.dma_start(
                out=slab[:], in_=wq_d[j].rearrange("(c p) m -> p c m", p=128)
            )
            for q in range(2):  # nq 512-chunk
                ps = pm.tile([128, 512], F32, name=f"psq{j}_{q}", tag="mm", bufs=4)
                for c in range(8):  # d_in chunk (contraction)
                    nc.tensor.matmul(
                        ps[:],
                        lhsT=slab[:, c, :],
                        rhs=xt[:, c, ds(q * 512, 512)],
                        start=(c == 0),
                        stop=(c == 7),
                    )
                nc.any.tensor_copy(qt[j][:, ds(q * 512, 512)], ps[:])
        t1.release()

        # ---- P2: kT[do, nkv] = sum_di Wk[di, do] * yT[di, nkv] ----------
        kt = [L.tile([128, NKV], BF16, name=f"kt{j}", tag="kt", bufs=8) for j in range(8)]
        for j in range(8):
            slab = t2.tile([128, 8, 128], BF16, name=f"wk{j}", tag="wk", bufs=3)
            nc.sync.dma_start(
                out=slab[:], in_=wk_d[j].rearrange("(c p) m -> p c m", p=128)
            )
            for q in range(4):  # nkv 512-chunk
                ps = pm.tile([128, 512], F32, name=f"psk{j}_{q}", tag="mm", bufs=4)
                for c in range(8):
                    nc.tensor.matmul(
                        ps[:],
                        lhsT=slab[:, c, :],
                        rhs=yt[:, c, ds(q * 512, 512)],
                        start=(c == 0),
                        stop=(c == 7),
                    )
                nc.any.tensor_copy(kt[j][:, ds(q * 512, 512)], ps[:])

        # ---- P3: v[nkv, do] = sum_di yT[di, nkv] * Wv[di, do] -----------
        vt = [L.tile([128, DIM], BF16, name=f"v{i}", tag="v", bufs=16) for i in range(16)]
        for i in range(16):  # nkv 128-tile
            for d in range(2):  # d_out 512-chunk
                ps = pm.tile([128, 512], F32, name=f"psv{i}_{d}", tag="mm", bufs=4)
                for c in range(8):
                    nc.tensor.matmul(
                        ps[:],
                        lhsT=yt[:, c, ds(i * 128, 128)],
                        rhs=wv[:, c, ds(d * 512, 512)],
                        start=(c == 0),
                        stop=(c == 7),
                    )
                nc.any.tensor_copy(vt[i][:, ds(d * 512, 512)], ps[:])
        t2.release()

        # ---- P4: expT[nkv, nq] = exp(sum_do kT[do,nkv] * qT[do,nq]) -----
        et = [L.tile([128, NQ_SHARD], BF16, name=f"e{i}", tag="et", bufs=16) for i in range(16)]
        for i in range(16):  # nkv 128-tile
            for q in range(2):  # nq 512-chunk
                ps = pm.tile([128, 512], F32, name=f"pse{i}_{q}", tag="mm", bufs=4)
                for j in range(8):  # d_out chunk (contraction)
                    nc.tensor.matmul(
                        ps[:],
                        lhsT=kt[j][:, ds(i * 128, 128)],
                        rhs=qt[j][:, ds(q * 512, 512)],
                        start=(j == 0),
                        stop=(j == 7),
                    )
                nc.scalar.activation(
                    et[i][:, ds(q * 512, 512)],
                    ps[:],
                    mybir.ActivationFunctionType.Exp,
                )

        # ---- P5: Z[nq] = sum_nkv expT[nkv, nq] (ones-stationary matmul) -
        ones = L.tile([128, 1], BF16, name="ones", bufs=1)
        nc.vector.memset(ones[:], 1.0)
        one_f32 = L.tile([1, 1], F32, name="one_f32", bufs=1)
        nc.vector.memset(one_f32[:], 1.0)
        zrec = L.tile([128, 8], F32, name="zrec", bufs=1)
        # Z lands as a [1, 512] psum row per nq-chunk; transpose each
        # 128-wide piece to a [128, 1] psum column with a K=1 matmul
        # (lhsT = row chunk, rhs = scalar 1).
        zps = pm.tile([128, 8], F32, name="zps", tag="zt", bufs=1)
        for q in range(2):
            psz = pm.tile([1, 512], F32, name=f"psz{q}", tag="z", bufs=2)
            for i in range(16):
                nc.tensor.matmul(
                    psz[:],
                    lhsT=ones[:],
                    rhs=et[i][:, ds(q * 512, 512)],
                    start=(i == 0),
                    stop=(i == 15),
                )
            zrow = L.tile([1, 512], F32, name=f"zrow{q}", tag="zrow", bufs=2)
            nc.any.tensor_copy(zrow[:], psz[:])
            for t in range(4):
                nc.tensor.matmul(
                    zps[:, ds(q * 4 + t, 1)],
                    lhsT=zrow[0:1, ds(t * 128, 128)],
                    rhs=one_f32[:],
                    start=True,
                    stop=True,
                )
        nc.vector.reciprocal(zrec[:], zps[:])

        # ---- P7: out[nq, do] = (sum_nkv expT[nkv,nq] * v[nkv,do]) / Z ---
        for t in range(8):  # nq 128-tile
            for d in range(2):  # d_out 512-chunk
                ps = pm.tile([128, 512], F32, name=f"pso{t}_{d}", tag="mm", bufs=4)
                for i in range(16):  # nkv contraction
                    nc.tensor.matmul(
                        ps[:],
                        lhsT=et[i][:, ds(t * 128, 128)],
                        rhs=vt[i][:, ds(d * 512, 512)],
                        start=(i == 0),
                        stop=(i == 15),
                    )
                ob = L.tile([128, 512], F32, name=f"o{t}_{d}", tag="o", bufs=3)
                nc.vector.tensor_scalar_mul(ob[:], ps[:], zrec[:, ds(t, 1)])
                nc.sync.dma_start(
                    out=out_d[ds(t * 128, 128), ds(d * 512, 512)], in_=ob[:]
                )
        pm.release()
        L.release()

    _split_sync_waits(nc)
    return nc


_NC_CACHE = {}


def _get_nc():
    if "nc" not in _NC_CACHE:
        _NC_CACHE["nc"] = build_attention_nc()
    return _NC_CACHE["nc"]


def make_in_maps(x, y, Wq, Wkv):
    """Host-side sharding + layout prep. Returns in_maps for cores 0-7."""
    scale = DIM ** (-0.5)
    wq_s = (np.asarray(Wq, np.float32) * scale).astype(NP_BF16)
    wkv = np.asarray(Wkv, np.float32)
    wk = wkv[:, :DIM].astype(NP_BF16)
    wv = wkv[:, DIM:].astype(NP_BF16)
    # column slabs [8, DIM, 128]
    wq_slabs = np.ascontiguousarray(wq_s.reshape(DIM, 8, 128).transpose(1, 0, 2))
    wk_slabs = np.ascontiguousarray(wk.reshape(DIM, 8, 128).transpose(1, 0, 2))

    x = np.asarray(x, np.float32)
    y = np.asarray(y, np.float32)
    in_maps = []
    for core in range(N_CORES):
        b, h = divmod(core, 2)
        xT = np.ascontiguousarray(
            x[b, h * NQ_SHARD : (h + 1) * NQ_SHARD, :].T
        ).astype(NP_BF16)
        yT = np.ascontiguousarray(y[b].T).astype(NP_BF16)
        in_maps.append(
            {"xT": xT, "yT": yT, "wq": wq_slabs, "wk": wk_slabs, "wv": wv}
        )
    return in_maps


def run_sharded(x, y, Wq, Wkv, trace=False, tmpdir=None):
    """Run the SPMD kernel; returns (full_output, BassKernelResults)."""
    nc = _get_nc()
    in_maps = make_in_maps(x, y, Wq, Wkv)
    res = run_bass_kernel_spmd(
        nc, in_maps, core_ids=list(range(N_CORES)), trace=trace, tmpdir=tmpdir
    )
    out = np.empty((B, NQ, DIM), np.float32)
    for core in range(N_CORES):
        b, h = divmod(core, 2)
        out[b, h * NQ_SHARD : (h + 1) * NQ_SHARD, :] = res.results[core]["out"]
    return out, res


def kernel(x, y, Wq, Wkv):
    out, _ = run_sharded(x, y, Wq, Wkv)
    return out


# revision 12
# speedup vs baseline: 1.3477x; 1.3477x over previous
"""Self-contained Trainium2 Bass kernel for single-head full-dim attention.

Reference computation (fp32 jax):
    q  = x @ Wq                      # [B, Nq, D]
    kv = y @ Wkv                     # [B, Nkv, 2D] -> k, v
    attn = softmax(q * D^-0.5 @ k^T) # [B, Nq, Nkv]
    out  = attn @ v                  # [B, Nq, D]
with B=4, Nq=Nkv=2048, D=1024.

Distribution: data parallel over 8 NeuronCores, shard = (batch b,
kv-half s).  Each core computes q for ALL 2048 queries of its batch
(cheap, duplicated across the pair), K/V for its 1024 keys, the
2048x1024 exp-score block, and the UNNORMALIZED output block
out'_s = exp(S_s) @ v_s plus the partial softmax denominator
Z_s = sum_k exp(S_s).  The host combines the two halves:
out = (out'_0 + out'_1) / (Z_0 + Z_1).  This avoids both collectives
and the (2x more expensive) duplicated K/V compute of a query-sharded
layout.

Layout trick: everything on-chip is computed transposed
([feature, token]) so the TensorEngine can contract along partitions
without any on-chip transposes.  The host pre-transposes x and y, folds
the D^-0.5 scale into Wq, and converts all matmul operands to bf16
(fp32 PSUM accumulation).  Softmax uses exp without max-subtraction
(scores ~ N(0,1) by construction; fp32 exp is safe) on the scalar
engine; Z is a ones-vector matmul.
"""

import numpy as np
import ml_dtypes

import concourse.bass as bass
import concourse.mybir as mybir
import concourse.tile as tile
from concourse.bass import ds
from concourse.bass_utils import run_bass_kernel_spmd

DIM = 1024
B = 4
NQ = 2048
NKV = 2048
N_CORES = 8
NKV_SHARD = 1024  # keys per core

BF16 = mybir.dt.bfloat16
F32 = mybir.dt.float32
NP_BF16 = ml_dtypes.bfloat16


def _split_sync_waits(nc, max_waits: int = 1):
    """walrus in this toolchain rejects instructions carrying more than one
    sem wait ("Too many sync wait commands").  Hoist extra waits onto
    preceding same-engine NOPs: the engine dispatches in order, so waiting
    just before the instruction is semantically identical (at worst it
    delays issue slightly)."""
    import bass_rust as _bass_rust

    for f in nc.m.functions:
        for bb in f.blocks:
            insts = list(bb.instructions)
            out = []
            changed = False
            for inst in insts:
                si = getattr(inst, "sync_info", None)
                waits = list(si.on_wait) if si is not None and si.on_wait else []
                if len(waits) > max_waits:
                    changed = True
                    extra, keep = waits[:-max_waits], waits[-max_waits:]
                    for k in range(0, len(extra), max_waits):
                        nop = mybir.InstNoOp(
                            name=f"{inst.name}_sw{k}", engine=inst.engine,
                            ins=[], outs=[],
                        )
                        nop.sync_info = _bass_rust.SyncInfo(
                            on_wait=extra[k : k + max_waits], on_update=[]
                        )
                        out.append(nop)
                    si.on_wait = keep
                    inst.sync_info = si
                out.append(inst)
            if changed:
                bb.instructions = out


def build_attention_nc():
    """Build the per-core Bass graph (identical on all 8 cores)."""
    nc = bass.Bass()

    # DRAM parameters (per-core shards, host-prepped layouts; all bf16
    # except the f32 outputs).
    xT_d = nc.declare_dram_parameter("xT", [DIM, NQ], BF16, isOutput=False)
    yT_d = nc.declare_dram_parameter("yT", [DIM, NKV_SHARD], BF16, isOutput=False)
    # wq/wk: column slabs: [do_chunk, d_in, 128], slab j = W[:, j*128:(j+1)*128]
    wq_d = nc.declare_dram_parameter("wq", [8, DIM, 128], BF16, isOutput=False)
    wk_d = nc.declare_dram_parameter("wk", [8, DIM, 128], BF16, isOutput=False)
    wv_d = nc.declare_dram_parameter("wv", [DIM, DIM], BF16, isOutput=False)
    out_d = nc.declare_dram_parameter("out", [NQ, DIM], F32, isOutput=True)
    # Z output in column-major tile layout: z[t*128 + p] = zout[p, t]
    z_d = nc.declare_dram_parameter("zout", [128, 16], F32, isOutput=True)

    with tile.TileContext(nc) as tc:
        # Long-lived pool: on-chip intermediates live to the end.
        L = tc.alloc_tile_pool(name="L", bufs=1)
        pm = tc.alloc_tile_pool(name="pm", bufs=1, space="PSUM")
        # Transient input pools, released once consumed.
        t2 = tc.alloc_tile_pool(name="t2", bufs=1)
        t1 = tc.alloc_tile_pool(name="t1", bufs=1)

        # ---- input DMAs (xt + wq slabs issue first: P1 gates on them) ---
        xt = t1.tile([128, 8, NQ], BF16, name="xt", bufs=1)
        nc.sync.dma_start(out=xt[:], in_=xT_d.rearrange("(c p) n -> p c n", p=128))

        # ---- P1: qT[do, nq] = sum_di Wq_s[di, do] * xT[di, nq] ----------
        qt = [L.tile([128, NQ], BF16, name=f"qt{j}", tag="qt", bufs=8) for j in range(8)]
        for j in range(8):  # d_out chunk
            slab = t1.tile([128, 8, 128], BF16, name=f"wq{j}", tag="wq", bufs=3)
            nc.sync.dma_start(
                out=slab[:], in_=wq_d[j].rearrange("(c p) m -> p c m", p=128)
            )
            for q in range(4):  # nq 512-chunk
                ps = pm.tile([128, 512], F32, name=f"psq{j}_{q}", tag="mm", bufs=4)
                for c in range(8):  # d_in chunk (contraction)
                    nc.tensor.matmul(
                        ps[:],
                        lhsT=slab[:, c, :],
                        rhs=xt[:, c, ds(q * 512, 512)],
                        start=(c == 0),
                        stop=(c == 7),
                    )
                nc.any.tensor_copy(qt[j][:, ds(q * 512, 512)], ps[:])

        # kv-side inputs (DMAs land while P1 computes)
        yt = t2.tile([128, 8, NKV_SHARD], BF16, name="yt", bufs=1)
        nc.sync.dma_start(out=yt[:], in_=yT_d.rearrange("(c p) n -> p c n", p=128))
        wv = t2.tile([128, 8, DIM], BF16, name="wv", bufs=1)
        nc.sync.dma_start(out=wv[:], in_=wv_d.rearrange("(c p) n -> p c n", p=128))
        t1.release()

        # ---- P2: kT[do, nkv] = sum_di Wk[di, do] * yT[di, nkv] ----------
        kt = [L.tile([128, NKV_SHARD], BF16, name=f"kt{j}", tag="kt", bufs=8) for j in range(8)]
        for j in range(8):
            slab = t2.tile([128, 8, 128], BF16, name=f"wk{j}", tag="wk", bufs=3)
            nc.sync.dma_start(
                out=slab[:], in_=wk_d[j].rearrange("(c p) m -> p c m", p=128)
            )
            for q in range(2):  # nkv 512-chunk
                ps = pm.tile([128, 512], F32, name=f"psk{j}_{q}", tag="mm", bufs=4)
                for c in range(8):
                    nc.tensor.matmul(
                        ps[:],
                        lhsT=slab[:, c, :],
                        rhs=yt[:, c, ds(q * 512, 512)],
                        start=(c == 0),
                        stop=(c == 7),
                    )
                nc.any.tensor_copy(kt[j][:, ds(q * 512, 512)], ps[:])

        # ---- P3: v[nkv, do] = sum_di yT[di, nkv] * Wv[di, do] -----------
        vt = [L.tile([128, DIM], BF16, name=f"v{i}", tag="v", bufs=8) for i in range(8)]
        for i in range(8):  # nkv 128-tile
            for d in range(2):  # d_out 512-chunk
                ps = pm.tile([128, 512], F32, name=f"psv{i}_{d}", tag="mm", bufs=4)
                for c in range(8):
                    nc.tensor.matmul(
                        ps[:],
                        lhsT=yt[:, c, ds(i * 128, 128)],
                        rhs=wv[:, c, ds(d * 512, 512)],
                        start=(c == 0),
                        stop=(c == 7),
                    )
                nc.any.tensor_copy(vt[i][:, ds(d * 512, 512)], ps[:])
        t2.release()

        # ---- P4: expT[nkv, nq] = exp(sum_do kT[do,nkv] * qT[do,nq]) -----
        et = [L.tile([128, NQ], BF16, name=f"e{i}", tag="et", bufs=8) for i in range(8)]
        for i in range(8):  # nkv 128-tile
            for q in range(4):  # nq 512-chunk
                ps = pm.tile([128, 512], F32, name=f"pse{i}_{q}", tag="mm", bufs=4)
                for j in range(8):  # d_out chunk (contraction)
                    nc.tensor.matmul(
                        ps[:],
                        lhsT=kt[j][:, ds(i * 128, 128)],
                        rhs=qt[j][:, ds(q * 512, 512)],
                        start=(j == 0),
                        stop=(j == 7),
                    )
                nc.scalar.activation(
                    et[i][:, ds(q * 512, 512)],
                    ps[:],
                    mybir.ActivationFunctionType.Exp,
                )

        # ---- P5: Z[nq] = sum_nkv expT[nkv, nq] (ones-stationary matmul) -
        ones = L.tile([128, 1], BF16, name="ones", bufs=1)
        nc.vector.memset(ones[:], 1.0)
        one_f32 = L.tile([1, 1], F32, name="one_f32", bufs=1)
        nc.vector.memset(one_f32[:], 1.0)
        # Z lands as [1, 512] psum rows; transpose each 128-wide piece to a
        # [128, 1] psum column with a K=1 matmul (lhsT = row chunk, rhs = 1).
        zps = pm.tile([128, 16], F32, name="zps", tag="zt", bufs=1)
        for q in range(4):
            psz = pm.tile([1, 512], F32, name=f"psz{q}", tag="z", bufs=2)
            for i in range(8):
                nc.tensor.matmul(
                    psz[:],
                    lhsT=ones[:],
                    rhs=et[i][:, ds(q * 512, 512)],
                    start=(i == 0),
                    stop=(i == 7),
                )
            zrow = L.tile([1, 512], F32, name=f"zrow{q}", tag="zrow", bufs=2)
            nc.any.tensor_copy(zrow[:], psz[:])
            for t in range(4):
                nc.tensor.matmul(
                    zps[:, ds(q * 4 + t, 1)],
                    lhsT=zrow[0:1, ds(t * 128, 128)],
                    rhs=one_f32[:],
                    start=True,
                    stop=True,
                )
        zcol = L.tile([128, 16], F32, name="zcol", bufs=1)
        nc.any.tensor_copy(zcol[:], zps[:])
        nc.sync.dma_start(out=z_d[:], in_=zcol[:])

        # ---- P7: out'[nq, do] = sum_nkv expT[nkv,nq] * v[nkv,do] --------
        for t in range(16):  # nq 128-tile
            for d in range(2):  # d_out 512-chunk
                ps = pm.tile([128, 512], F32, name=f"pso{t}_{d}", tag="mm", bufs=4)
                for i in range(8):  # nkv contraction
                    nc.tensor.matmul(
                        ps[:],
                        lhsT=et[i][:, ds(t * 128, 128)],
                        rhs=vt[i][:, ds(d * 512, 512)],
                        start=(i == 0),
                        stop=(i == 7),
                    )
                ob = L.tile([128, 512], F32, name=f"o{t}_{d}", tag="o", bufs=3)
                nc.any.tensor_copy(ob[:], ps[:])
                nc.sync.dma_start(
                    out=out_d[ds(t * 128, 128), ds(d * 512, 512)], in_=ob[:]
                )
        pm.release()
        L.release()

    _split_sync_waits(nc)
    return nc


_NC_CACHE = {}


def _get_nc():
    if "nc" not in _NC_CACHE:
        _NC_CACHE["nc"] = build_attention_nc()
    return _NC_CACHE["nc"]


def make_in_maps(x, y, Wq, Wkv):
    """Host-side sharding + layout prep. Returns in_maps for cores 0-7."""
    scale = DIM ** (-0.5)
    wq_s = (np.asarray(Wq, np.float32) * scale).astype(NP_BF16)
    wkv = np.asarray(Wkv, np.float32)
    wk = wkv[:, :DIM].astype(NP_BF16)
    wv = wkv[:, DIM:].astype(NP_BF16)
    # column slabs [8, DIM, 128]
    wq_slabs = np.ascontiguousarray(wq_s.reshape(DIM, 8, 128).transpose(1, 0, 2))
    wk_slabs = np.ascontiguousarray(wk.reshape(DIM, 8, 128).transpose(1, 0, 2))

    x = np.asarray(x, np.float32)
    y = np.asarray(y, np.float32)
    in_maps = []
    for core in range(N_CORES):
        b, s = divmod(core, 2)
        xT = np.ascontiguousarray(x[b].T).astype(NP_BF16)
        yT = np.ascontiguousarray(
            y[b, s * NKV_SHARD : (s + 1) * NKV_SHARD, :].T
        ).astype(NP_BF16)
        in_maps.append(
            {"xT": xT, "yT": yT, "wq": wq_slabs, "wk": wk_slabs, "wv": wv}
        )
    return in_maps


def run_sharded(x, y, Wq, Wkv, trace=False, tmpdir=None):
    """Run the SPMD kernel; returns (full_output, BassKernelResults)."""
    nc = _get_nc()
    in_maps = make_in_maps(x, y, Wq, Wkv)
    res = run_bass_kernel_spmd(
        nc, in_maps, core_ids=list(range(N_CORES)), trace=trace, tmpdir=tmpdir
    )
    out = np.empty((B, NQ, DIM), np.float32)
    for b in range(B):
        r0, r1 = res.results[2 * b], res.results[2 * b + 1]
        num = r0["out"] + r1["out"]
        z = (r0["zout"] + r1["zout"]).T.reshape(NQ)
        out[b] = num / z[:, None]
    return out, res


def kernel(x, y, Wq, Wkv):
    out, _ = run_sharded(x, y, Wq, Wkv)
    return out
